# revision 2
# baseline (speedup 1.0000x reference)
"""LMS adaptive filter (BaseFilter) on 8 TRN2 NeuronCores.

Algorithm per (batch b, frame f): 64-tap LMS over 416 sequential steps.
  e_t   = d[b, 256f + 32 + t] - sum_k w[k] * x[256f + t + k]
  w     = clip(w + MU * e_t * x[256f + t : +64], +-65535)
The clip is essential: mu*|x_win|^2 ~ 3.2 > 2 makes the recursion
unstable, so w rides the clip rails and the rails keep all float
implementations shadowing each other. f32 required (bf16 diverges).

Sharding: 4096 frames split 512/core (both batches on every core) ->
1024 independent sequences/core = 8 groups x 128 partitions, organized
as 2 quads (quad q = batch q, slabs 0-3).

Per step (sustained-rate measured on HW):
  DVE   8x dot: scalar_tensor_tensor + accum (145ns/pair)
  Act   8x e_g = ns_g + d_t (bias AP), written to ET[:, t, g]
  DVE   2x TMP_q = MUX_window * e-bcast (TT256, stride-0 AP, 335ns)
  DVE   2x W' = W + TMP (TT256, 335ns)
  Pool  2x clip W' in place (TS256, 353ns)
This replaces the 8 narrow scalar-AP updates (235ns each in situ: the
per-partition scalar port read costs ~60ns/instr) with 2+2 wide TTs.
d_est is NOT computed on-chip: host does d_est = d - e (exact, since
reference defines e = d - d_est).
"""

import numpy as np

HOP = 256
FRAMELEN = 512
K = 64
WD = 32
MU = 0.05
WMIN, WMAX = -65535.0, 65535.0
B = 2
F = 4096
NC = 8
F_LOC = F // NC              # 512 frames per core
S = (FRAMELEN - K) - WD      # 416 sequential steps
TSTART = (FRAMELEN - HOP) - WD  # 224: first step kept for frames >= 1
TAIL = S - TSTART            # 192 output elements per frame >= 1
SPAN = HOP * (F_LOC - 1) + FRAMELEN  # 131328: x/d elements per core shard
CORE_STRIDE = HOP * F_LOC    # 131072
OUT_LEN = (FRAMELEN - K) + (F - 1) * TAIL  # 786688

NBUF = 2                     # NS/PROD buffer depth

_CACHE = {}


def _build():
    import concourse.bacc as bacc
    import concourse.tile as tile
    from concourse import mybir
    import concourse.bass as bass

    f32 = mybir.dt.float32
    AluOp = mybir.AluOpType

    nc = bacc.Bacc("TRN2", target_bir_lowering=False)
    x_in = nc.dram_tensor("x", [SPAN], f32, kind="ExternalInput")
    d_in = nc.dram_tensor("d", [B, SPAN], f32, kind="ExternalInput")
    # e only; d_est = d - e on host.  [b][f_local][j], j <-> t = TSTART + j
    out_e = nc.dram_tensor("out_e", [B, F_LOC, TAIL], f32,
                           kind="ExternalOutput")
    out_head = nc.dram_tensor("out_head", [B, TSTART], f32,
                              kind="ExternalOutput")

    with tile.TileContext(nc) as tc:
        with tc.tile_pool(name="p", bufs=1) as pool:
            XF = pool.tile([128, 4, FRAMELEN], f32)    # x frames (slab fg)
            XFMU = pool.tile([128, 4, FRAMELEN], f32)  # MU * x frames
            DB = pool.tile([128, B, 4, S], f32)        # d at step offsets
            # weights: quad q covers groups 4q..4q+3 (batch q, slabs 0-3)
            WQ = [[pool.tile([128, 4, K], f32, name=f"WQ{q}_{i}",
                             tag=f"wq{q}_{i}") for i in range(2)]
                  for q in range(2)]
            TMPQ = [[pool.tile([128, 4, K], f32, name=f"TMP{q}_{i}",
                               tag=f"tmp{q}_{i}") for i in range(2)]
                    for q in range(2)]
            # e history: ET[:, t, g]  (serves e-bcast reads AND output)
            ET = pool.tile([128, S, 8], f32, name="ET", tag="et")
            NS = [[pool.tile([128, 1], f32, name=f"NS{g}_{i}",
                             tag=f"n{g}_{i}") for i in range(NBUF)]
                  for g in range(8)]
            PROD = [[pool.tile([128, K], f32, name=f"PROD{g}_{i}",
                               tag=f"p{g}_{i}") for i in range(NBUF)]
                    for g in range(8)]
            EOUT = pool.tile([128, 8, TAIL], f32, name="EOUT", tag="eout")

            # partition p, slab fg  ->  frame f_local = fg*128 + p
            for q in range(2):
                nc.vector.memset(WQ[q][0][:], 0.0)
            for fg in range(4):
                nc.sync.dma_start(
                    XF[:, fg, :],
                    bass.AP(tensor=x_in, offset=HOP * 128 * fg,
                            ap=[[HOP, 128], [1, FRAMELEN]]),
                )
                nc.vector.tensor_scalar_mul(XFMU[:, fg, :], XF[:, fg, :], MU)
                for b in range(B):
                    nc.sync.dma_start(
                        DB[:, b, fg, :],
                        bass.AP(tensor=d_in,
                                offset=b * SPAN + HOP * 128 * fg + WD,
                                ap=[[HOP, 128], [1, S]]),
                    )

            def emit_dot(t, i, q, j):
                g = 4 * q + j
                nc.vector.scalar_tensor_tensor(
                    out=PROD[g][i][:], in0=WQ[q][t % 2][:, j, :],
                    scalar=-1.0, in1=XF[:, j, t:t + K],
                    op0=AluOp.mult, op1=AluOp.mult,
                    accum_out=NS[g][i][:, 0:1],
                )

            def emit_act(t, i, q, j):
                g = 4 * q + j
                nc.scalar.activation(
                    out=ET[:, t, g:g + 1], in_=NS[g][i][:, 0:1],
                    func=mybir.ActivationFunctionType.Identity,
                    bias=DB[:, q, j, t:t + 1], scale=1.0,
                )

            def emit_upd(t, q):
                cur, nxt = WQ[q][t % 2], WQ[q][(t + 1) % 2]
                tmp = TMPQ[q][t % 2]
                xa = XFMU[:]
                xquad = bass.AP(tensor=xa.tensor, offset=xa.offset + t,
                                ap=[list(xa.ap[0]), [FRAMELEN, 4], [1, K]])
                ea = ET[:]
                ebc = bass.AP(tensor=ea.tensor,
                              offset=ea.offset + 8 * t + 4 * q,
                              ap=[list(ea.ap[0]), [1, 4], [0, K]])
                nc.vector.tensor_tensor(out=tmp[:], in0=xquad, in1=ebc,
                                        op=AluOp.mult)
                nc.vector.tensor_tensor(out=nxt[:], in0=cur[:], in1=tmp[:],
                                        op=AluOp.add)
                nc.gpsimd.tensor_scalar(
                    out=nxt[:], in0=nxt[:],
                    scalar1=WMAX, scalar2=WMIN,
                    op0=AluOp.min, op1=AluOp.max,
                )

            for t in range(S):
                i = t % NBUF
                for q in range(2):
                    for j in range(4):
                        emit_dot(t, i, q, j)
                for q in range(2):
                    for j in range(4):
                        emit_act(t, i, q, j)
                for q in range(2):
                    emit_upd(t, q)

            # repack e history (stride-8) into contiguous EOUT, then one DMA
            for g in range(8):
                ea = ET[:]
                src = bass.AP(tensor=ea.tensor,
                              offset=ea.offset + 8 * TSTART + g,
                              ap=[list(ea.ap[0]), [8, TAIL]])
                nc.vector.tensor_copy(out=EOUT[:, g, :], in_=src)
            ea = EOUT[:]
            nc.sync.dma_start(
                bass.AP(tensor=out_e, offset=0,
                        ap=[[TAIL, 128], [F_LOC * TAIL, B], [128 * TAIL, 4],
                            [1, TAIL]]),
                bass.AP(tensor=ea.tensor, offset=ea.offset,
                        ap=[list(ea.ap[0]), [4 * TAIL, B], [TAIL, 4],
                            [1, TAIL]]),
            )
            # head: frame 0 of this core (only core 0's matters), groups b*4
            eh = ET[:]
            for b in range(B):
                nc.sync.dma_start(
                    bass.AP(tensor=out_head, offset=b * TSTART,
                            ap=[[TSTART, 1], [1, TSTART]]),
                    bass.AP(tensor=eh.tensor, offset=eh.offset + 4 * b,
                            ap=[[eh.ap[0][0], 1], [8, TSTART]]),
                )
    nc.finalize()
    return nc


def _get_nc():
    if "nc" not in _CACHE:
        _CACHE["nc"] = _build()
    return _CACHE["nc"]


def run_shards(d, x, trace=False, **kw):
    from concourse.bass_utils import run_bass_kernel_spmd

    nc = _get_nc()
    in_maps = []
    for c in range(NC):
        lo = c * CORE_STRIDE
        in_maps.append({
            "x": np.ascontiguousarray(x[lo:lo + SPAN], dtype=np.float32),
            "d": np.ascontiguousarray(d[:, lo:lo + SPAN], dtype=np.float32),
        })
    return run_bass_kernel_spmd(nc, in_maps, core_ids=list(range(NC)),
                                trace=trace, **kw)


def assemble(results, d):
    es = np.stack([r["out_e"] for r in results])     # (8, B, 512, 192)
    head = results[0]["out_head"]                    # (B, 224)

    # d windows: dwin[b, f, t] = d[b, 256 f + WD + t], t in [0, S)
    idx = HOP * np.arange(F)[:, None] + WD + np.arange(S)[None, :]
    dwin = d[:, idx]                                 # (B, F, S)

    def ola(head_v, main_v):
        # head_v: (B, TSTART) frame-0 steps t<TSTART
        # main_v: (B, F, TAIL) steps t in [TSTART, S) for every frame
        o = np.zeros((B, OUT_LEN), np.float32)
        o[:, WD:WD + TSTART] = head_v
        o[:, WD + TSTART:FRAMELEN - K] = main_v[:, 0]
        o[:, FRAMELEN - K:] = main_v[:, 1:].reshape(B, -1)
        return o

    e_main = es.transpose(1, 0, 2, 3).reshape(B, F, TAIL)
    e_out = ola(head, e_main)
    dest_out = ola(dwin[:, 0, :TSTART] - head,
                   dwin[:, :, TSTART:] - e_main)
    return dest_out, e_out


def kernel(d, x):
    d = np.asarray(d, dtype=np.float32)
    x = np.asarray(x, dtype=np.float32)
    res = run_shards(d, x)
    return assemble(res.results, d)


# revision 3
# speedup vs baseline: 1.0523x; 1.0523x over previous
"""LMS adaptive filter (BaseFilter) on 8 TRN2 NeuronCores.

Algorithm per (batch b, frame f): 64-tap LMS over 416 sequential steps.
  e_t   = d[b, 256f + 32 + t] - sum_k w[k] * x[256f + t + k]
  w     = clip(w + MU * e_t * x[256f + t : +64], +-65535)
The clip is essential: mu*|x_win|^2 ~ 3.2 > 2 makes the recursion
unstable, so w rides the clip rails and the rails keep all float
implementations shadowing each other. f32 required (bf16 diverges).

Sharding: 4096 frames split 512/core (both batches on every core) ->
1024 independent sequences/core = 8 groups x 128 partitions, organized
as 2 quads (quad q = batch q, slabs 0-3).

V3: everything except the clip runs on the Vector engine, as two
phase-shifted quad units per step:
  unit q: [4x dot STT+accum -> NSQ slices] [e: 1x TT4 d_t + ns]
          [TMP: TT256 mu*x-window x e-bcast] [W': TT256 add]
  Pool:   clip W' in place (TS256), covered by the other quad's unit
This removes Act from the chain (e is same-engine, in-order -> no
semaphore) and the only cross-engine edge is Pool-clip -> next dots,
hidden by ~1.3us of other-quad work.  d_est is not computed on-chip:
host does d_est = d - e (exact: reference defines e = d - d_est).
"""

import numpy as np

HOP = 256
FRAMELEN = 512
K = 64
WD = 32
MU = 0.05
WMIN, WMAX = -65535.0, 65535.0
B = 2
F = 4096
NC = 8
F_LOC = F // NC              # 512 frames per core
S = (FRAMELEN - K) - WD      # 416 sequential steps
TSTART = (FRAMELEN - HOP) - WD  # 224: first step kept for frames >= 1
TAIL = S - TSTART            # 192 output elements per frame >= 1
SPAN = HOP * (F_LOC - 1) + FRAMELEN  # 131328: x/d elements per core shard
CORE_STRIDE = HOP * F_LOC    # 131072
OUT_LEN = (FRAMELEN - K) + (F - 1) * TAIL  # 786688

NBUF = 2                     # NSQ/PROD buffer depth
DOT_MODE = "narrow"          # "narrow": 4x STT+accum; "quad": TT256+RED4

_CACHE = {}


def _build():
    import concourse.bacc as bacc
    import concourse.tile as tile
    from concourse import mybir
    import concourse.bass as bass

    f32 = mybir.dt.float32
    AluOp = mybir.AluOpType

    nc = bacc.Bacc("TRN2", target_bir_lowering=False)
    x_in = nc.dram_tensor("x", [SPAN], f32, kind="ExternalInput")
    d_in = nc.dram_tensor("d", [B, SPAN], f32, kind="ExternalInput")
    # e only; d_est = d - e on host.  [b][f_local][j], j <-> t = TSTART + j
    out_e = nc.dram_tensor("out_e", [B, F_LOC, TAIL], f32,
                           kind="ExternalOutput")
    out_head = nc.dram_tensor("out_head", [B, TSTART], f32,
                              kind="ExternalOutput")

    with tile.TileContext(nc) as tc:
        with tc.tile_pool(name="p", bufs=1) as pool:
            XF = pool.tile([128, 4, FRAMELEN], f32)    # x frames (slab fg)
            XFMU = pool.tile([128, 4, FRAMELEN], f32)  # MU * x frames
            DB = pool.tile([128, B, 4, S], f32)        # d at step offsets
            # weights: quad q covers groups 4q..4q+3 (batch q, slabs 0-3)
            WQ = [[pool.tile([128, 4, K], f32, name=f"WQ{q}_{i}",
                             tag=f"wq{q}_{i}") for i in range(2)]
                  for q in range(2)]
            TMPQ = [[pool.tile([128, 4, K], f32, name=f"TMP{q}_{i}",
                               tag=f"tmp{q}_{i}") for i in range(2)]
                    for q in range(2)]
            # e history: ET[:, t, g]  (serves e-bcast reads AND output)
            ET = pool.tile([128, S, 8], f32, name="ET", tag="et")
            NSQ = [[pool.tile([128, 4], f32, name=f"NSQ{q}_{i}",
                              tag=f"n{q}_{i}") for i in range(NBUF)]
                   for q in range(2)]
            PROD = [[pool.tile([128, 4, K], f32, name=f"PROD{q}_{i}",
                               tag=f"p{q}_{i}") for i in range(NBUF)]
                    for q in range(2)]
            EOUT = pool.tile([128, 8, TAIL], f32, name="EOUT", tag="eout")

            # partition p, slab fg  ->  frame f_local = fg*128 + p
            for q in range(2):
                nc.vector.memset(WQ[q][0][:], 0.0)
            for fg in range(4):
                nc.sync.dma_start(
                    XF[:, fg, :],
                    bass.AP(tensor=x_in, offset=HOP * 128 * fg,
                            ap=[[HOP, 128], [1, FRAMELEN]]),
                )
                nc.vector.tensor_scalar_mul(XFMU[:, fg, :], XF[:, fg, :], MU)
                for b in range(B):
                    nc.sync.dma_start(
                        DB[:, b, fg, :],
                        bass.AP(tensor=d_in,
                                offset=b * SPAN + HOP * 128 * fg + WD,
                                ap=[[HOP, 128], [1, S]]),
                    )

            def xquad(base, t):
                xa = base[:]
                return bass.AP(tensor=xa.tensor, offset=xa.offset + t,
                               ap=[list(xa.ap[0]), [FRAMELEN, 4], [1, K]])

            def emit_unit(t, q):
                i = t % NBUF
                cur, nxt = WQ[q][t % 2], WQ[q][(t + 1) % 2]
                # dots
                if DOT_MODE == "narrow":
                    for j in range(4):
                        nc.vector.scalar_tensor_tensor(
                            out=PROD[q][i][:, j, :],
                            in0=cur[:, j, :], scalar=-1.0,
                            in1=XF[:, j, t:t + K],
                            op0=AluOp.mult, op1=AluOp.mult,
                            accum_out=NSQ[q][i][:, j:j + 1],
                        )
                else:
                    nc.vector.tensor_tensor(
                        out=PROD[q][i][:], in0=cur[:], in1=xquad(XF, t),
                        op=AluOp.mult)
                    nc.vector.tensor_reduce(
                        out=NSQ[q][i][:], in_=PROD[q][i][:],
                        axis=mybir.AxisListType.X, op=AluOp.add,
                        negate=True)
                # e (same engine, in-order: no cross-engine latency)
                da = DB[:]
                dt = bass.AP(tensor=da.tensor,
                             offset=da.offset + (q * 4) * S + t,
                             ap=[list(da.ap[0]), [S, 4]])
                nc.vector.tensor_tensor(
                    out=ET[:, t, 4 * q:4 * q + 4], in0=dt,
                    in1=NSQ[q][i][:], op=AluOp.add)
                # update: TMP = (mu x) * e_bcast ; W' = W + TMP
                ea = ET[:]
                ebc = bass.AP(tensor=ea.tensor,
                              offset=ea.offset + 8 * t + 4 * q,
                              ap=[list(ea.ap[0]), [1, 4], [0, K]])
                tmp = TMPQ[q][t % 2]
                nc.vector.tensor_tensor(out=tmp[:], in0=xquad(XFMU, t),
                                        in1=ebc, op=AluOp.mult)
                nc.vector.tensor_tensor(out=nxt[:], in0=cur[:], in1=tmp[:],
                                        op=AluOp.add)
                # clip on Pool, covered by the other quad's unit
                nc.gpsimd.tensor_scalar(
                    out=nxt[:], in0=nxt[:],
                    scalar1=WMAX, scalar2=WMIN,
                    op0=AluOp.min, op1=AluOp.max,
                )

            for t in range(S):
                emit_unit(t, 0)
                emit_unit(t, 1)

            # repack e history (stride-8) into contiguous EOUT, then one DMA
            for g in range(8):
                ea = ET[:]
                src = bass.AP(tensor=ea.tensor,
                              offset=ea.offset + 8 * TSTART + g,
                              ap=[list(ea.ap[0]), [8, TAIL]])
                nc.vector.tensor_copy(out=EOUT[:, g, :], in_=src)
            ea = EOUT[:]
            nc.sync.dma_start(
                bass.AP(tensor=out_e, offset=0,
                        ap=[[TAIL, 128], [F_LOC * TAIL, B], [128 * TAIL, 4],
                            [1, TAIL]]),
                bass.AP(tensor=ea.tensor, offset=ea.offset,
                        ap=[list(ea.ap[0]), [4 * TAIL, B], [TAIL, 4],
                            [1, TAIL]]),
            )
            # head: frame 0 of this core (only core 0's matters), groups b*4
            eh = ET[:]
            for b in range(B):
                nc.sync.dma_start(
                    bass.AP(tensor=out_head, offset=b * TSTART,
                            ap=[[TSTART, 1], [1, TSTART]]),
                    bass.AP(tensor=eh.tensor, offset=eh.offset + 4 * b,
                            ap=[[eh.ap[0][0], 1], [8, TSTART]]),
                )
    nc.finalize()
    return nc


def _get_nc():
    if "nc" not in _CACHE:
        _CACHE["nc"] = _build()
    return _CACHE["nc"]


def run_shards(d, x, trace=False, **kw):
    from concourse.bass_utils import run_bass_kernel_spmd

    nc = _get_nc()
    in_maps = []
    for c in range(NC):
        lo = c * CORE_STRIDE
        in_maps.append({
            "x": np.ascontiguousarray(x[lo:lo + SPAN], dtype=np.float32),
            "d": np.ascontiguousarray(d[:, lo:lo + SPAN], dtype=np.float32),
        })
    return run_bass_kernel_spmd(nc, in_maps, core_ids=list(range(NC)),
                                trace=trace, **kw)


def assemble(results, d):
    es = np.stack([r["out_e"] for r in results])     # (8, B, 512, 192)
    head = results[0]["out_head"]                    # (B, 224)

    # d windows: dwin[b, f, t] = d[b, 256 f + WD + t], t in [0, S)
    idx = HOP * np.arange(F)[:, None] + WD + np.arange(S)[None, :]
    dwin = d[:, idx]                                 # (B, F, S)

    def ola(head_v, main_v):
        # head_v: (B, TSTART) frame-0 steps t<TSTART
        # main_v: (B, F, TAIL) steps t in [TSTART, S) for every frame
        o = np.zeros((B, OUT_LEN), np.float32)
        o[:, WD:WD + TSTART] = head_v
        o[:, WD + TSTART:FRAMELEN - K] = main_v[:, 0]
        o[:, FRAMELEN - K:] = main_v[:, 1:].reshape(B, -1)
        return o

    e_main = es.transpose(1, 0, 2, 3).reshape(B, F, TAIL)
    e_out = ola(head, e_main)
    dest_out = ola(dwin[:, 0, :TSTART] - head,
                   dwin[:, :, TSTART:] - e_main)
    return dest_out, e_out


def kernel(d, x):
    d = np.asarray(d, dtype=np.float32)
    x = np.asarray(x, dtype=np.float32)
    res = run_shards(d, x)
    return assemble(res.results, d)
